# revision 9
# baseline (speedup 1.0000x reference)
"""Causal self-attention on 8 Trainium2 NeuronCores.

Problem: x [4, 2048, 1024], 16 heads of dim 64, causal softmax attention,
output projection. Sharding: core c = 2*b + g handles batch b and head
group g (8 heads = 512 of the 1024 QKV columns). Each core computes its
partial y contribution (att_out @ Wp-rows of its head group); the host sums
the two partials per batch and adds the output bias.

On-device layout (per core):
  xT   [1024, 2048]  x[b] transposed (d on partitions)
  qT/kT [512, 2048]  projections transposed (head-col on partitions)
  v'   [2048, 520]   v in natural layout, 65 cols per head (64 data + ones)
  S~^T tiles [128k, 512q] scores transposed, exp'd, causal-masked
  out' [65, 512]     per (head, q-chunk): rows 0-63 att_out^T, row 64 sums
  attT [512, 2048]   normalized attention output, transposed
  yT   [1024, 2048]  output partial, transposed

All matmuls use float32r (TF32-like fast fp32 mode, ~1e-4 rel err).
Softmax skips the max-subtraction (scores are O(5), exp is safe in fp32)
so the row-sum comes free as a ones-column in the AV matmul.
"""

import numpy as np

import concourse.bass as bass
import concourse.mybir as mybir
import concourse.tile as tile
from concourse import bacc
from concourse.bass_utils import run_bass_kernel_spmd

F32 = mybir.dt.float32
F32R = mybir.dt.float32r

B, T, D = 4, 2048, 1024
H, HD = 16, 64
NCORES = 8
HPC = 8          # heads per core
GC = 512         # head-group columns per core (HPC * HD)
KC = 128         # contraction chunk
NQ = 512         # moving-dim chunk (max for 4-byte dtypes)
NT = T // NQ     # 4 q/t chunks
NKB = T // KC    # 16 k blocks
SCALE = 1.0 / np.sqrt(HD)

_CACHE = {}


def _build():
    nc = bacc.Bacc("TRN2")
    xT = nc.declare_dram_parameter("xT", [D, T], F32R, isOutput=False)
    wq = nc.declare_dram_parameter("wq", [D, GC], F32R, isOutput=False)
    wk = nc.declare_dram_parameter("wk", [D, GC], F32R, isOutput=False)
    wv = nc.declare_dram_parameter("wv", [D, GC], F32R, isOutput=False)
    wp = nc.declare_dram_parameter("wp", [GC, D], F32R, isOutput=False)
    bqk = nc.declare_dram_parameter("bqk", [128, 8], F32, isOutput=False)
    bv = nc.declare_dram_parameter("bv", [1, GC], F32, isOutput=False)
    masks = nc.declare_dram_parameter("masks", [4, 128, NQ], F32R, isOutput=False)
    yT = nc.declare_dram_parameter("yT", [D, T], F32, isOutput=True)

    Ident = mybir.ActivationFunctionType.Identity
    Exp = mybir.ActivationFunctionType.Exp

    with tile.TileContext(nc) as tc:
        with tc.tile_pool(name="qkt", bufs=1) as qkt_pool, \
             tc.tile_pool(name="vv", bufs=1) as v_pool, \
             tc.tile_pool(name="const", bufs=1) as const_pool:

            # ---- constants ----
            mask_sb = []
            for d in range(4):
                mt = const_pool.tile([128, NQ], F32R, tag=f"mask{d}")
                nc.sync.dma_start(out=mt, in_=masks[d, :, :])
                mask_sb.append(mt)
            bqk_sb = const_pool.tile([128, 8], F32, tag="bqk")
            nc.sync.dma_start(out=bqk_sb, in_=bqk[:, :])
            bv_sb = const_pool.tile([128, GC], F32, tag="bv")
            bv_bcast = bass.AP(tensor=bv[0:1, :].tensor, offset=0,
                               ap=[[0, 128], [1, GC]])
            nc.gpsimd.dma_start(out=bv_sb, in_=bv_bcast)

            qT = [qkt_pool.tile([128, T], F32R, tag=f"qT{m}", name=f"qT{m}") for m in range(4)]
            kT = [qkt_pool.tile([128, T], F32R, tag=f"kT{m}", name=f"kT{m}") for m in range(4)]
            vv = [v_pool.tile([128, HPC * 65], F32R, tag=f"v{t}", name=f"v{t}") for t in range(NKB)]

            # ---- phases A+B: projections ----
            with tc.tile_pool(name="x", bufs=1) as x_pool:
                xs = []
                for kc in range(D // KC):
                    xt = x_pool.tile([128, T], F32R, tag=f"x{kc}", name=f"x{kc}")
                    nc.sync.dma_start(out=xt, in_=xT[kc * KC:(kc + 1) * KC, :])
                    xs.append(xt)

                # phase A: q/k projections (transposed)
                with tc.tile_pool(name="wqk", bufs=1) as wqk_pool, \
                     tc.tile_pool(name="psA", bufs=4, space="PSUM") as psA:
                    wqs, wks = [], []
                    for kc in range(D // KC):
                        wqt = wqk_pool.tile([128, GC], F32R, tag=f"wq{kc}", name=f"wq{kc}")
                        nc.sync.dma_start(out=wqt, in_=wq[kc * KC:(kc + 1) * KC, :])
                        wqs.append(wqt)
                        wkt = wqk_pool.tile([128, GC], F32R, tag=f"wk{kc}", name=f"wk{kc}")
                        nc.sync.dma_start(out=wkt, in_=wk[kc * KC:(kc + 1) * KC, :])
                        wks.append(wkt)

                    for dst, ws, bcol in ((qT, wqs, 0), (kT, wks, 4)):
                        for m in range(4):
                            for n in range(NT):
                                pt = psA.tile([128, NQ], F32)
                                for kc in range(D // KC):
                                    nc.tensor.matmul(
                                        pt,
                                        lhsT=ws[kc][:, m * 128:(m + 1) * 128],
                                        rhs=xs[kc][:, n * NQ:(n + 1) * NQ],
                                        start=(kc == 0), stop=(kc == D // KC - 1))
                                nc.scalar.activation(
                                    out=dst[m][:, n * NQ:(n + 1) * NQ], in_=pt,
                                    func=Ident, bias=bqk_sb[:, bcol + m:bcol + m + 1])

                # phase B: v (natural layout, ones columns interleaved)
                with tc.tile_pool(name="wv", bufs=1) as wv_pool, \
                     tc.tile_pool(name="psB", bufs=4, space="PSUM") as psB:
                    wvs = []
                    for kc in range(D // KC):
                        wvt = wv_pool.tile([128, GC], F32R, tag=f"wv{kc}", name=f"wv{kc}")
                        nc.sync.dma_start(out=wvt, in_=wv[kc * KC:(kc + 1) * KC, :])
                        wvs.append(wvt)

                    for t in range(NKB):
                        # pre-fill with 1.0; the per-head data adds overwrite
                        # everything except each head's ones column
                        nc.vector.memset(vv[t].bitcast(F32), 1.0)
                        pt = psB.tile([128, GC], F32)
                        for kc in range(D // KC):
                            nc.tensor.matmul(
                                pt,
                                lhsT=xs[kc][:, t * 128:(t + 1) * 128],
                                rhs=wvs[kc],
                                start=(kc == 0), stop=(kc == D // KC - 1))
                        # scatter into v' (65-col stride per head)
                        for h in range(HPC):
                            nc.vector.tensor_add(
                                out=vv[t][:, h * 65:h * 65 + 64],
                                in0=pt[:, h * 64:(h + 1) * 64],
                                in1=bv_sb[:, h * 64:(h + 1) * 64])

            # ---- phase C: attention per head ----
            with tc.tile_pool(name="att", bufs=1) as att_pool:
              attT = [att_pool.tile([128, T], F32R, tag=f"attT{m}", name=f"attT{m}")
                      for m in range(4)]
              with tc.tile_pool(name="psS", bufs=4, space="PSUM") as psS, \
                 tc.tile_pool(name="psAV", bufs=2, space="PSUM") as psAV, \
                 tc.tile_pool(name="es", bufs=20) as es_pool, \
                 tc.tile_pool(name="esm", bufs=6) as esm_pool, \
                 tc.tile_pool(name="rs", bufs=2) as rs_pool, \
                 tc.tile_pool(name="rb", bufs=2) as rb_pool:
                for h in range(HPC):
                    mt = h // 2
                    po = (h % 2) * 64
                    kh = kT[mt]
                    qh = qT[mt]
                    for j in range(NT):
                        nmm = 4 * j + 4
                        es_tiles = []
                        for i in range(nmm):
                            ps = psS.tile([128, NQ], F32)
                            nc.tensor.matmul(
                                ps,
                                lhsT=kh[po:po + 64, i * KC:(i + 1) * KC],
                                rhs=qh[po:po + 64, j * NQ:(j + 1) * NQ],
                                start=True, stop=True)
                            es = es_pool.tile([128, NQ], F32R, tag="es")
                            nc.scalar.activation(out=es, in_=ps, func=Exp,
                                                 scale=float(SCALE))
                            if i >= 4 * j:
                                esm = esm_pool.tile([128, NQ], F32R, tag="esm")
                                nc.vector.tensor_mul(out=esm, in0=es,
                                                     in1=mask_sb[i - 4 * j])
                                es = esm
                            es_tiles.append(es)
                        pav = psAV.tile([65, NQ], F32)
                        for i in range(nmm):
                            nc.tensor.matmul(
                                pav,
                                lhsT=vv[i][:, h * 65:(h + 1) * 65],
                                rhs=es_tiles[i],
                                start=(i == 0), stop=(i == nmm - 1))
                        # normalize: recip of sums row, broadcast, multiply
                        rst = rs_pool.tile([32, NQ], F32, tag="rs")
                        nc.vector.reciprocal(out=rst[0:1, :], in_=pav[64:65, :])
                        rbt = rb_pool.tile([64, NQ], F32, tag="rb")
                        nc.gpsimd.partition_broadcast(rbt, rst[0:1, :])
                        nc.vector.tensor_mul(
                            out=attT[mt][po:po + 64, j * NQ:(j + 1) * NQ],
                            in0=pav[0:64, :], in1=rbt)

              # ---- phase D: output projection ----
              with tc.tile_pool(name="wp", bufs=1) as wp_pool, \
                 tc.tile_pool(name="psD", bufs=4, space="PSUM") as psD, \
                 tc.tile_pool(name="y", bufs=4) as y_pool:
                wps = []
                for kc in range(4):
                    wpt = wp_pool.tile([128, D], F32R, tag=f"wp{kc}")
                    nc.sync.dma_start(out=wpt, in_=wp[kc * KC:(kc + 1) * KC, :])
                    wps.append(wpt)
                for m in range(D // KC):
                    for n in range(NT):
                        pt = psD.tile([128, NQ], F32)
                        for kc in range(4):
                            nc.tensor.matmul(
                                pt,
                                lhsT=wps[kc][:, m * 128:(m + 1) * 128],
                                rhs=attT[kc][:, n * NQ:(n + 1) * NQ],
                                start=(kc == 0), stop=(kc == 3))
                        yt = y_pool.tile([128, NQ], F32, tag="y")
                        nc.scalar.activation(out=yt, in_=pt, func=Ident)
                        nc.sync.dma_start(
                            out=yT[m * 128:(m + 1) * 128, n * NQ:(n + 1) * NQ],
                            in_=yt)

    nc.compile()
    return nc


def _masks_np():
    kk = np.arange(128, dtype=np.int64)[:, None]
    qq = np.arange(NQ, dtype=np.int64)[None, :]
    return np.stack(
        [(kk <= qq - 128 * d).astype(np.float32) for d in range(4)], axis=0)


def _in_maps(inputs):
    x = np.asarray(inputs["x"], dtype=np.float32)
    Wq = np.asarray(inputs["Wq"], dtype=np.float32)
    Wk = np.asarray(inputs["Wk"], dtype=np.float32)
    Wv = np.asarray(inputs["Wv"], dtype=np.float32)
    Wp = np.asarray(inputs["Wp"], dtype=np.float32)
    bq = np.asarray(inputs["bq"], dtype=np.float32)
    bk = np.asarray(inputs["bk"], dtype=np.float32)
    bv = np.asarray(inputs["bv"], dtype=np.float32)
    masks = _masks_np()
    maps = []
    for c in range(NCORES):
        b, g = c // 2, c % 2
        s = slice(g * GC, (g + 1) * GC)
        bq_g = bq[s].reshape(4, 128).T
        bk_g = bk[s].reshape(4, 128).T
        maps.append({
            "xT": np.ascontiguousarray(x[b].T),
            "wq": np.ascontiguousarray(Wq[:, s]),
            "wk": np.ascontiguousarray(Wk[:, s]),
            "wv": np.ascontiguousarray(Wv[:, s]),
            "wp": np.ascontiguousarray(Wp[s, :]),
            "bqk": np.ascontiguousarray(np.hstack([bq_g, bk_g])),
            "bv": np.ascontiguousarray(bv[s][None, :]),
            "masks": masks,
        })
    return maps


def _gather(results, bp):
    y = np.empty((B, T, D), dtype=np.float32)
    for b in range(B):
        acc = results[2 * b]["yT"] + results[2 * b + 1]["yT"]
        y[b] = acc.T + bp[None, :]
    return y


def _run(inputs, **kwargs):
    if "nc" not in _CACHE:
        _CACHE["nc"] = _build()
    res = run_bass_kernel_spmd(_CACHE["nc"], _in_maps(inputs),
                               core_ids=list(range(NCORES)), **kwargs)
    bp = np.asarray(inputs["bp"], dtype=np.float32)
    return _gather(res.results, bp), res


def kernel(**inputs) -> np.ndarray:
    out, _ = _run(inputs)
    return out


# revision 13
# speedup vs baseline: 1.0361x; 1.0361x over previous
"""Causal self-attention on 8 Trainium2 NeuronCores.

Problem: x [4, 2048, 1024], 16 heads of dim 64, causal softmax attention,
output projection. Sharding: core c = 2*b + g handles batch b and head
group g (8 heads = 512 of the 1024 QKV columns). Each core computes its
partial y contribution (att_out @ Wp-rows of its head group); the host sums
the two partials per batch and adds the output bias.

On-device layout (per core):
  xT   [1024, 2048]  x[b] transposed (d on partitions)
  qT/kT [512, 2048]  projections transposed (head-col on partitions)
  v'   [2048, 520]   v in natural layout, 65 cols per head (64 data + ones)
  S~^T tiles [128k, 512q] scores transposed, exp'd, causal-masked
  out' [65, 512]     per (head, q-chunk): rows 0-63 att_out^T, row 64 sums
  attT [512, 2048]   normalized attention output, transposed
  yT   [1024, 2048]  output partial, transposed

All matmuls use float32r (TF32-like fast fp32 mode, ~1e-4 rel err).
Softmax skips the max-subtraction (scores are O(5), exp is safe in fp32)
so the row-sum comes free as a ones-column in the AV matmul.
"""

import numpy as np

import concourse.bass as bass
import concourse.mybir as mybir
import concourse.tile as tile
from concourse import bacc
from concourse.bass_utils import run_bass_kernel_spmd

F32 = mybir.dt.float32
F32R = mybir.dt.float32r

B, T, D = 4, 2048, 1024
H, HD = 16, 64
NCORES = 8
HPC = 8          # heads per core
GC = 512         # head-group columns per core (HPC * HD)
KC = 128         # contraction chunk
NQ = 512         # moving-dim chunk (max for 4-byte dtypes)
NT = T // NQ     # 4 q/t chunks
NKB = T // KC    # 16 k blocks
SCALE = 1.0 / np.sqrt(HD)

_CACHE = {}


def _build():
    nc = bacc.Bacc("TRN2")
    xT = nc.declare_dram_parameter("xT", [D, T], F32R, isOutput=False)
    wq = nc.declare_dram_parameter("wq", [D, GC], F32R, isOutput=False)
    wk = nc.declare_dram_parameter("wk", [D, GC], F32R, isOutput=False)
    wv = nc.declare_dram_parameter("wv", [D, GC], F32R, isOutput=False)
    wp = nc.declare_dram_parameter("wp", [GC, D], F32R, isOutput=False)
    bqk = nc.declare_dram_parameter("bqk", [128, 8], F32, isOutput=False)
    bv = nc.declare_dram_parameter("bv", [1, GC], F32, isOutput=False)
    masks = nc.declare_dram_parameter("masks", [4, 128, NQ], F32R, isOutput=False)
    yT = nc.declare_dram_parameter("yT", [D, T], F32, isOutput=True)

    Ident = mybir.ActivationFunctionType.Identity
    Exp = mybir.ActivationFunctionType.Exp

    with tile.TileContext(nc) as tc:
        with tc.tile_pool(name="qkt", bufs=1) as qkt_pool, \
             tc.tile_pool(name="vv", bufs=1) as v_pool, \
             tc.tile_pool(name="const", bufs=1) as const_pool:

            # ---- constants ----
            mask_sb = []
            for d in range(4):
                mt = const_pool.tile([128, NQ], F32R, tag=f"mask{d}")
                nc.sync.dma_start(out=mt, in_=masks[d, :, :])
                mask_sb.append(mt)
            bqk_sb = const_pool.tile([128, 8], F32, tag="bqk")
            nc.sync.dma_start(out=bqk_sb, in_=bqk[:, :])
            bv_sb = const_pool.tile([128, GC], F32, tag="bv")
            bv_bcast = bass.AP(tensor=bv[0:1, :].tensor, offset=0,
                               ap=[[0, 128], [1, GC]])
            nc.gpsimd.dma_start(out=bv_sb, in_=bv_bcast)

            qT = [qkt_pool.tile([128, T], F32R, tag=f"qT{m}", name=f"qT{m}") for m in range(4)]
            kT = [qkt_pool.tile([128, T], F32R, tag=f"kT{m}", name=f"kT{m}") for m in range(4)]
            vv = [v_pool.tile([128, HPC * 65], F32R, tag=f"v{t}", name=f"v{t}") for t in range(NKB)]

            # ---- phases A+B: projections ----
            with tc.tile_pool(name="x", bufs=1) as x_pool:
                xs = []
                for kc in range(D // KC):
                    xt = x_pool.tile([128, T], F32R, tag=f"x{kc}", name=f"x{kc}")
                    nc.sync.dma_start(out=xt, in_=xT[kc * KC:(kc + 1) * KC, :])
                    xs.append(xt)

                # phase A: q/k projections (transposed)
                with tc.tile_pool(name="wqk", bufs=1) as wqk_pool, \
                     tc.tile_pool(name="psA", bufs=4, space="PSUM") as psA:
                    wqs, wks = [], []
                    for kc in range(D // KC):
                        wqt = wqk_pool.tile([128, GC], F32R, tag=f"wq{kc}", name=f"wq{kc}")
                        nc.sync.dma_start(out=wqt, in_=wq[kc * KC:(kc + 1) * KC, :])
                        wqs.append(wqt)
                        wkt = wqk_pool.tile([128, GC], F32R, tag=f"wk{kc}", name=f"wk{kc}")
                        nc.sync.dma_start(out=wkt, in_=wk[kc * KC:(kc + 1) * KC, :])
                        wks.append(wkt)

                    for dst, ws, bcol in ((qT, wqs, 0), (kT, wks, 4)):
                        for m in range(4):
                            for n in range(NT):
                                pt = psA.tile([128, NQ], F32)
                                for kc in range(D // KC):
                                    nc.tensor.matmul(
                                        pt,
                                        lhsT=ws[kc][:, m * 128:(m + 1) * 128],
                                        rhs=xs[kc][:, n * NQ:(n + 1) * NQ],
                                        start=(kc == 0), stop=(kc == D // KC - 1))
                                nc.scalar.activation(
                                    out=dst[m][:, n * NQ:(n + 1) * NQ], in_=pt,
                                    func=Ident, bias=bqk_sb[:, bcol + m:bcol + m + 1])

                # phase B: v (natural layout, ones columns interleaved)
                with tc.tile_pool(name="wv", bufs=1) as wv_pool, \
                     tc.tile_pool(name="psB", bufs=4, space="PSUM") as psB:
                    wvs = []
                    for kc in range(D // KC):
                        wvt = wv_pool.tile([128, GC], F32R, tag=f"wv{kc}", name=f"wv{kc}")
                        nc.sync.dma_start(out=wvt, in_=wv[kc * KC:(kc + 1) * KC, :])
                        wvs.append(wvt)

                    for t in range(NKB):
                        # pre-fill with 1.0; the per-head data adds overwrite
                        # everything except each head's ones column
                        nc.vector.memset(vv[t].bitcast(F32), 1.0)
                        pt = psB.tile([128, GC], F32)
                        for kc in range(D // KC):
                            nc.tensor.matmul(
                                pt,
                                lhsT=xs[kc][:, t * 128:(t + 1) * 128],
                                rhs=wvs[kc],
                                start=(kc == 0), stop=(kc == D // KC - 1))
                        # scatter into v' (65-col stride per head)
                        for h in range(HPC):
                            nc.vector.tensor_add(
                                out=vv[t][:, h * 65:h * 65 + 64],
                                in0=pt[:, h * 64:(h + 1) * 64],
                                in1=bv_sb[:, h * 64:(h + 1) * 64])

            # ---- phase C: attention per head ----
            with tc.tile_pool(name="att", bufs=1) as att_pool:
              attT = [att_pool.tile([128, T], F32R, tag=f"attT{m}", name=f"attT{m}")
                      for m in range(4)]
              with tc.tile_pool(name="psS", bufs=4, space="PSUM") as psS, \
                 tc.tile_pool(name="psAV", bufs=2, space="PSUM") as psAV, \
                 tc.tile_pool(name="es", bufs=20) as es_pool, \
                 tc.tile_pool(name="esm", bufs=6) as esm_pool, \
                 tc.tile_pool(name="rs", bufs=2) as rs_pool, \
                 tc.tile_pool(name="rb", bufs=2) as rb_pool:
                for h in range(HPC):
                    mt = h // 2
                    po = (h % 2) * 64
                    kh = kT[mt]
                    qh = qT[mt]
                    for j in range(NT):
                        nmm = 4 * j + 4
                        es_tiles = []
                        for i in range(nmm):
                            ps = psS.tile([128, NQ], F32)
                            nc.tensor.matmul(
                                ps,
                                lhsT=kh[po:po + 64, i * KC:(i + 1) * KC],
                                rhs=qh[po:po + 64, j * NQ:(j + 1) * NQ],
                                start=True, stop=True)
                            es = es_pool.tile([128, NQ], F32R, tag="es")
                            nc.scalar.activation(out=es, in_=ps, func=Exp,
                                                 scale=float(SCALE))
                            if i >= 4 * j:
                                esm = esm_pool.tile([128, NQ], F32R, tag="esm")
                                nc.vector.tensor_mul(out=esm, in0=es,
                                                     in1=mask_sb[i - 4 * j])
                                es = esm
                            es_tiles.append(es)
                        pav = psAV.tile([65, NQ], F32)
                        for i in range(nmm):
                            nc.tensor.matmul(
                                pav,
                                lhsT=vv[i][:, h * 65:(h + 1) * 65],
                                rhs=es_tiles[i],
                                start=(i == 0), stop=(i == nmm - 1))
                        # normalize: recip of sums row, broadcast, multiply
                        rsc = rs_pool.tile([32, NQ], F32, tag="rsc")
                        nc.vector.tensor_copy(out=rsc[0:1, :], in_=pav[64:65, :])
                        rst = rs_pool.tile([32, NQ], F32, tag="rs")
                        nc.vector.reciprocal_approx_fast(out=rst[0:1, :], in_=rsc[0:1, :])
                        rbt = rb_pool.tile([64, NQ], F32, tag="rb")
                        nc.gpsimd.partition_broadcast(rbt, rst[0:1, :])
                        nc.vector.tensor_mul(
                            out=attT[mt][po:po + 64, j * NQ:(j + 1) * NQ],
                            in0=pav[0:64, :], in1=rbt)

              # ---- phase D: output projection ----
              with tc.tile_pool(name="wp", bufs=1) as wp_pool, \
                 tc.tile_pool(name="psD", bufs=4, space="PSUM") as psD, \
                 tc.tile_pool(name="y", bufs=4) as y_pool:
                wps = []
                for kc in range(4):
                    wpt = wp_pool.tile([128, D], F32R, tag=f"wp{kc}")
                    nc.sync.dma_start(out=wpt, in_=wp[kc * KC:(kc + 1) * KC, :])
                    wps.append(wpt)
                for m in range(D // KC):
                    for n in range(NT):
                        pt = psD.tile([128, NQ], F32)
                        for kc in range(4):
                            nc.tensor.matmul(
                                pt,
                                lhsT=wps[kc][:, m * 128:(m + 1) * 128],
                                rhs=attT[kc][:, n * NQ:(n + 1) * NQ],
                                start=(kc == 0), stop=(kc == 3))
                        yt = y_pool.tile([128, NQ], F32, tag="y")
                        nc.scalar.activation(out=yt, in_=pt, func=Ident)
                        nc.sync.dma_start(
                            out=yT[m * 128:(m + 1) * 128, n * NQ:(n + 1) * NQ],
                            in_=yt)

    nc.compile()
    return nc


def _masks_np():
    kk = np.arange(128, dtype=np.int64)[:, None]
    qq = np.arange(NQ, dtype=np.int64)[None, :]
    return np.stack(
        [(kk <= qq - 128 * d).astype(np.float32) for d in range(4)], axis=0)


def _in_maps(inputs):
    x = np.asarray(inputs["x"], dtype=np.float32)
    Wq = np.asarray(inputs["Wq"], dtype=np.float32)
    Wk = np.asarray(inputs["Wk"], dtype=np.float32)
    Wv = np.asarray(inputs["Wv"], dtype=np.float32)
    Wp = np.asarray(inputs["Wp"], dtype=np.float32)
    bq = np.asarray(inputs["bq"], dtype=np.float32)
    bk = np.asarray(inputs["bk"], dtype=np.float32)
    bv = np.asarray(inputs["bv"], dtype=np.float32)
    masks = _masks_np()
    maps = []
    for c in range(NCORES):
        b, g = c // 2, c % 2
        s = slice(g * GC, (g + 1) * GC)
        bq_g = bq[s].reshape(4, 128).T
        bk_g = bk[s].reshape(4, 128).T
        maps.append({
            "xT": np.ascontiguousarray(x[b].T),
            "wq": np.ascontiguousarray(Wq[:, s]),
            "wk": np.ascontiguousarray(Wk[:, s]),
            "wv": np.ascontiguousarray(Wv[:, s]),
            "wp": np.ascontiguousarray(Wp[s, :]),
            "bqk": np.ascontiguousarray(np.hstack([bq_g, bk_g])),
            "bv": np.ascontiguousarray(bv[s][None, :]),
            "masks": masks,
        })
    return maps


def _gather(results, bp):
    y = np.empty((B, T, D), dtype=np.float32)
    for b in range(B):
        acc = results[2 * b]["yT"] + results[2 * b + 1]["yT"]
        y[b] = acc.T + bp[None, :]
    return y


def _run(inputs, **kwargs):
    if "nc" not in _CACHE:
        _CACHE["nc"] = _build()
    res = run_bass_kernel_spmd(_CACHE["nc"], _in_maps(inputs),
                               core_ids=list(range(NCORES)), **kwargs)
    bp = np.asarray(inputs["bp"], dtype=np.float32)
    return _gather(res.results, bp), res


def kernel(**inputs) -> np.ndarray:
    out, _ = _run(inputs)
    return out
